# revision 23
# baseline (speedup 1.0000x reference)
"""E3Conv Trainium2 kernel: 8-core SPMD, dst-partitioned edges.

Strategy: sort edges by dst; core i owns nodes [1250i,1250(i+1)) and all edges
into them (no all-reduce needed). Per core: node-MLP replicated, SBUF-resident
bf16 gather table for Ai (recip folded into one-hot scatter weights), radial
MLP + tensor-product as K=512 matmuls per edge tile, PSUM-accumulated
one-hot matmul scatter-mean. Engine balance: PE matmuls, DVE fused
PSUM-multiply TTs, Act silu/copies, Pool gathers.
"""
import sys, os
sys.path.insert(0, "/opt/trn_rl_repo")
import numpy as np

import concourse.bass as bass
import concourse.tile as tile
from concourse import bacc, mybir
from concourse import bass_utils
from concourse.masks import make_identity

P = 128
N_NODES, N_EDGES, N_GRAPHS = 10000, 131072, 64
N_CORES, NPC, N_WIN = 8, 1250, 10
MAX_RADIUS, N_BASIS = 4.0, 10
STEP = MAX_RADIUS / (N_BASIS + 1)
VCENTERS = np.linspace(0.0, MAX_RADIUS, N_BASIS + 2)[1:-1].astype(np.float32)
F32, BF16, I16 = mybir.dt.float32, mybir.dt.bfloat16, mybir.dt.int16
AF = mybir.ActivationFunctionType
ALU = mybir.AluOpType
NCH = 79  # node chunks of 128 (79*128 = 10112 >= 10000)
NF = 19   # geometry features: 10 basis + 3 sh1(u) + 5 sh2 + ones


def _build_consts(fW4):
    s3 = 3.0 ** 0.5
    W4p = np.zeros((512, 224), np.float32)
    offs = {0: 0, 1: 1024, 2: 1536}
    wbase = {0: 0, 1: 16, 2: 24}
    scale_l = {0: 1.0 / 64, 1: s3 / 64, 2: 1.0 / 64}
    for l, mo in enumerate((16, 8, 4)):
        for u in range(8):
            for v in range(8):
                for wl in range(mo):
                    col = offs[l] + (u * 8 + v) * mo + wl
                    w = wbase[l] + wl
                    W4p[np.arange(64) * 8 + v, w * 8 + u] = fW4[:, col] * scale_l[l]
    Sel = np.zeros((4, 64, 128), np.float32)
    for q in range(4):
        for r in range(128):
            Sel[q, 16 * q + r // 8, r] = 1.0
    # L2A: tm0 (w=0..13) -> l0 slots 0..13 ; L2B: tm1 (w=14..27) ->
    # l0 slots 14,15 ; l1 slots 16+(w-16)*3+m ; l2 slots 40+(w-24)*5+k
    L2A = np.zeros((112, 60), np.float32)
    L2B = np.zeros((112, 60), np.float32)
    for r in range(112):
        L2A[r, r // 8] = 1.0
        w = 14 + r // 8
        if w < 16:
            L2B[r, w] = 1.0
        elif w < 24:
            for m in range(3):
                L2B[r, 16 + (w - 16) * 3 + m] = 1.0
        else:
            for k in range(5):
                L2B[r, 40 + (w - 24) * 5 + k] = 1.0
    # HSb: bt19 rows (10 basis, u xyz, 5 sh2, ones) -> shs rows
    # [0:16 ones | 16:40 l1 = u comps | 40:60 l2 comps]
    HSb = np.zeros((19, 60), np.float32)
    HSb[18, 0:16] = 1.0
    for w in range(8):
        for m in range(3):
            HSb[10 + m, 16 + w * 3 + m] = 1.0
    for w in range(4):
        for k in range(5):
            HSb[13 + k, 40 + w * 5 + k] = 1.0
    return W4p, Sel, L2A, L2B, HSb


def _merge_hs(HSb, fW1p):
    HS = np.zeros((19, 124), np.float32)
    HS[0:10, 0:64] = fW1p
    HS[:, 64:124] = HSb
    return HS


def _host_prep(inputs):
    pos = np.asarray(inputs["pos"], np.float32)
    A = np.asarray(inputs["A"]).astype(np.int64)
    batch = np.asarray(inputs["batch"]).astype(np.int64)
    esrc = np.asarray(inputs["edge_src"]).astype(np.int64)
    edst = np.asarray(inputs["edge_dst"]).astype(np.int64)
    shifts = np.asarray(inputs["edge_shifts"], np.float32)
    cell = np.asarray(inputs["cell"], np.float32)
    counts = np.bincount(edst, minlength=N_NODES).astype(np.float32)
    recipc = 1.0 / np.maximum(counts, 1.0)
    cpn = cell[batch].reshape(N_NODES, 9)
    order = np.argsort(edst, kind="stable")
    wins_all, W_CH = [], 0
    for ci in range(N_CORES):
        lo = ci * NPC
        m = order[(edst[order] >= lo) & (edst[order] < lo + NPC)]
        wins = []
        for w in range(N_WIN):
            wlo = lo + w * P
            whi = min(lo + (w + 1) * P, lo + NPC)
            wm = m[(edst[m] >= wlo) & (edst[m] < whi)]
            wins.append(wm)
            W_CH = max(W_CH, (len(wm) + P - 1) // P)
        wins_all.append(wins)
    if W_CH % 2:
        W_CH += 1
    C_TOT = N_WIN * W_CH
    E = C_TOT * P
    onehotA = np.zeros((10, NCH * P), np.float32)
    onehotA[A, np.arange(N_NODES)] = 1.0
    recip_pad = np.concatenate([recipc, np.ones(N_WIN * P * N_CORES, np.float32)])
    per_core = []
    for ci in range(N_CORES):
        idx = np.zeros(E, np.int64)
        valid = np.zeros(E, bool)
        dstloc = np.full(E, -1.0, np.float32)
        for w in range(N_WIN):
            wm = wins_all[ci][w]
            s = w * W_CH * P
            idx[s:s + len(wm)] = wm
            valid[s:s + len(wm)] = True
            dstloc[s:s + len(wm)] = (edst[wm] - ci * NPC - w * P).astype(np.float32)
        src = np.where(valid, esrc[idx], 0)
        dst = np.where(valid, edst[idx], 0)
        sh = np.where(valid[:, None], shifts[idx], np.float32(1.0))
        geom = np.concatenate([pos[src], pos[dst], sh, cpn[src]], 1)  # [E,18]
        geom_pl = np.ascontiguousarray(
            np.transpose(geom.reshape(C_TOT, P, 18), (1, 2, 0)).reshape(P, 18 * C_TOT))

        def wrap(ix):
            wr = ix.astype(np.int16).reshape(-1, 16).T  # [16, E/16]
            return np.ascontiguousarray(np.tile(wr, (8, 1)))
        # one-hot scatter weights carry the scatter-mean reciprocal
        ohm = (dstloc.reshape(C_TOT, P, 1) ==
               np.arange(P, dtype=np.float32)[None, None, :]).astype(np.float32)
        for w in range(N_WIN):
            rw = recip_pad[ci * NPC + w * P: ci * NPC + (w + 1) * P]
            ohm[w * W_CH:(w + 1) * W_CH] *= rw[None, None, :]
        oh_pl = np.ascontiguousarray(
            np.transpose(ohm, (1, 0, 2)).reshape(P, C_TOT * P))
        per_core.append(dict(geom_pl=geom_pl, oh_pl=oh_pl,
                             idx_src=wrap(src), idx_dst=wrap(dst)))
    return per_core, onehotA, W_CH, C_TOT, E


def _build_bass(W_CH, C_TOT, E, consts):
    TILE_CH = W_CH // 2
    NT = C_TOT // TILE_CH
    ET = TILE_CH * P
    NIW = E // 16
    nc = bacc.Bacc("TRN2", target_bir_lowering=False, debug=False,
                   num_devices=N_CORES)

    def din(name, shape, dt=F32):
        return nc.dram_tensor(name, shape, dt, kind="ExternalInput").ap()

    geom_d = din("geom_pl", [P, 18 * C_TOT])
    ohm_d = din("oh_pl", [P, C_TOT * P], BF16)
    isrc_d = din("idx_src", [P, NIW], I16)
    idst_d = din("idx_dst", [P, NIW], I16)
    ohA_d = din("onehotA", [10, NCH * P], BF16)
    TA_d = din("TA", [64, 10], BF16)
    W2_d = din("fit_W2", [64, 32], BF16)
    W3_d = din("fit_W3", [32, 8], BF16)
    HS_d = din("HS", [19, 124], BF16)
    fW2_d = din("fc_W2p", [64, 64], BF16)
    fW3_d = din("fc_W3p", [64, 4 * 128], BF16)
    W4p_d = din("W4p", [128, 4 * 224], BF16)
    L2A_d = din("L2A", [112, 60], BF16)
    L2B_d = din("L2B", [112, 60], BF16)
    cv_d = din("cvec", [P, 16])
    out_d = nc.dram_tensor("out", [N_WIN * P, 60], F32, kind="ExternalOutput").ap()

    C = C_TOT
    with tile.TileContext(nc) as tc:
        with tc.tile_pool(name="const", bufs=1) as cp, \
             tc.tile_pool(name="sb", bufs=2) as sp, \
             tc.tile_pool(name="big", bufs=1) as bp, \
             tc.tile_pool(name="ps", bufs=2, space="PSUM") as ps, \
             tc.tile_pool(name="pc", bufs=1, space="PSUM") as pc, \
             tc.tile_pool(name="pf", bufs=1, space="PSUM") as pf, \
             tc.tile_pool(name="pw", bufs=1, space="PSUM") as pw:
            ident = cp.tile([P, P], F32)
            make_identity(nc, ident[:])
            identb = cp.tile([P, P], BF16)
            nc.vector.tensor_copy(identb[:], ident[:])

            def load_const(dram, shape, dt=F32):
                t = cp.tile(shape, dt, tag=dram.tensor.name)
                nc.sync.dma_start(t[:], dram[:])
                return t
            TA = load_const(TA_d, [64, 10], BF16)
            W2 = load_const(W2_d, [64, 32], BF16)
            W3 = load_const(W3_d, [32, 8], BF16)
            HSt = load_const(HS_d, [19, 124], BF16)
            fW2 = load_const(fW2_d, [64, 64], BF16)
            fW3 = load_const(fW3_d, [64, 4 * 128], BF16)
            W4pt = load_const(W4p_d, [128, 4 * 224], BF16)
            L2At = load_const(L2A_d, [112, 60], BF16)
            L2Bt = load_const(L2B_d, [112, 60], BF16)
            cv = load_const(cv_d, [P, 16])
            ohA = bp.tile([10, NCH * P], BF16)
            nc.sync.dma_start(ohA[:], ohA_d[:])
            isrc = bp.tile([P, NIW], I16)
            nc.sync.dma_start(isrc[:], isrc_d[:])
            idst = bp.tile([P, NIW], I16)
            nc.sync.dma_start(idst[:], idst_d[:])

            # ---- node MLP degenerates to a 10-row type table (input depends
            # only on atom type); expand per 128-node chunk into the SBUF
            # gather table (node n -> partition n%128, rank n//128, x16) ----
            s1 = sp.tile([64, 10], BF16, tag="ns1")
            nc.scalar.activation(s1[:], TA[:], AF.Silu)
            h2t = pw.tile([32, 10], F32, tag="w")
            nc.tensor.matmul(h2t[:], W2[:], s1[:], start=True, stop=True)
            s2 = sp.tile([32, 10], BF16, tag="ns2")
            nc.scalar.activation(s2[:], h2t[:], AF.Silu)
            atp = pw.tile([10, 8], F32, tag="w")
            nc.tensor.matmul(atp[:], s2[:], W3[:], start=True, stop=True)
            AiTab = sp.tile([10, 8], BF16, tag="nat")
            nc.scalar.copy(AiTab[:], atp[:])
            Tsb = bp.tile([P, NCH * P], BF16)
            j = 0
            while j * 896 < NCH * P:
                s = j * 896
                n = min(896, NCH * P - s)
                ncc = n // P
                aiT = pf.tile([P, 7 * 8], F32, tag="f")
                for c in range(ncc):
                    nc.tensor.matmul(aiT[:, c * 8:(c + 1) * 8],
                                     ohA[:, s + c * P:s + (c + 1) * P],
                                     AiTab[:], start=True, stop=True)
                f16a = sp.tile([P, 7 * 8], BF16, tag="f16a")
                nc.scalar.copy(f16a[:, 0:ncc * 8], aiT[:, 0:ncc * 8])
                nc.vector.tensor_copy(
                    Tsb[:, s:s + n].rearrange("p (k r v) -> p k r v", v=8, r=16),
                    f16a[:, 0:ncc * 8].rearrange("p (k v) -> p k v", v=8)
                    .unsqueeze(2).to_broadcast([P, ncc, 16, 8]))
                j += 1

            # ---------------- geometry (plane layout, whole E) ----------------
            gm = bp.tile([P, 18 * C], F32)
            nc.sync.dma_start(gm[:], geom_d[:])
            g3 = gm[:].rearrange("p (f c) -> p f c", f=18)
            tmp9 = bp.tile([P, 9 * C], F32)
            nc.vector.tensor_tensor(
                out=tmp9[:].rearrange("p (i j c) -> p i j c", i=3, j=3),
                in0=gm[:, 9 * C:18 * C].rearrange("p (i j c) -> p i j c", i=3, j=3),
                in1=gm[:, 6 * C:9 * C].rearrange("p (i c) -> p i c", i=3)
                    .unsqueeze(2).to_broadcast([P, 3, 3, C]),
                op=ALU.mult)
            sv = bp.tile([P, 3 * C], F32)
            nc.vector.tensor_tensor(out=sv[:], in0=tmp9[:, 0:3 * C],
                                    in1=tmp9[:, 3 * C:6 * C], op=ALU.add)
            nc.vector.tensor_tensor(out=sv[:], in0=sv[:],
                                    in1=tmp9[:, 6 * C:9 * C], op=ALU.add)
            ev = bp.tile([P, 3 * C], F32)
            nc.vector.tensor_tensor(out=ev[:], in0=g3[:, 3:6].rearrange("p f c -> p (f c)"),
                                    in1=g3[:, 0:3].rearrange("p f c -> p (f c)"),
                                    op=ALU.subtract)
            nc.vector.tensor_tensor(out=ev[:], in0=ev[:], in1=sv[:], op=ALU.add)
            sq = bp.tile([P, 3 * C], F32)
            nc.gpsimd.tensor_tensor(out=sq[:], in0=ev[:], in1=ev[:], op=ALU.mult)
            ln2 = bp.tile([P, C], F32)
            nc.vector.tensor_tensor(out=ln2[:], in0=sq[:, 0:C], in1=sq[:, C:2 * C],
                                    op=ALU.add)
            nc.vector.tensor_tensor(out=ln2[:], in0=ln2[:], in1=sq[:, 2 * C:3 * C],
                                    op=ALU.add)
            ln = bp.tile([P, C], F32)
            nc.scalar.activation(ln[:], ln2[:], AF.Sqrt)
            rl = bp.tile([P, C], F32)
            nc.vector.reciprocal(rl[:], ln[:])
            u = bp.tile([P, 3 * C], F32)
            nc.vector.tensor_tensor(
                out=u[:].rearrange("p (f c) -> p f c", f=3),
                in0=ev[:].rearrange("p (f c) -> p f c", f=3),
                in1=rl[:].unsqueeze(1).to_broadcast([P, 3, C]), op=ALU.mult)
            usq = bp.tile([P, 3 * C], F32)
            nc.gpsimd.tensor_tensor(out=usq[:], in0=u[:], in1=u[:], op=ALU.mult)
            # feature planes: f-major [basis10 | u 3 | sh2 5 | ones]
            gf = bp.tile([P, NF * C], BF16)
            dt2 = bp.tile([P, 10 * C], F32)
            for b in range(N_BASIS):
                nc.scalar.activation(dt2[:, b * C:(b + 1) * C], ln[:], AF.Square,
                                     bias=cv[:, b:b + 1],
                                     scale=cv[:, 10:11])
            nc.scalar.activation(gf[:, 0:10 * C], dt2[:], AF.Exp,
                                 scale=cv[:, 11:12])
            nc.vector.tensor_copy(gf[:, 10 * C:13 * C], u[:])
            t1 = bp.tile([P, C], F32)
            nc.scalar.mul(t1[:], u[:, 2 * C:3 * C], cv[:, 12:13])       # sqrt15*uz
            nc.gpsimd.tensor_tensor(out=gf[:, 13 * C:14 * C], in0=u[:, 0:C],
                                    in1=t1[:], op=ALU.mult)     # m0
            nc.gpsimd.tensor_tensor(out=gf[:, 16 * C:17 * C], in0=u[:, C:2 * C],
                                    in1=t1[:], op=ALU.mult)     # m3
            nc.scalar.mul(t1[:], u[:, 0:C], cv[:, 12:13])               # sqrt15*ux
            nc.gpsimd.tensor_tensor(out=gf[:, 14 * C:15 * C], in0=u[:, C:2 * C],
                                    in1=t1[:], op=ALU.mult)     # m1
            t2 = bp.tile([P, C], F32)
            nc.vector.tensor_tensor(out=t2[:], in0=usq[:, 0:C],
                                    in1=usq[:, 2 * C:3 * C], op=ALU.add)
            nc.scalar.mul(t2[:], t2[:], cv[:, 13:14])
            t3 = bp.tile([P, C], F32)
            nc.scalar.mul(t3[:], usq[:, C:2 * C], cv[:, 14:15])
            nc.vector.tensor_tensor(out=gf[:, 15 * C:16 * C], in0=t3[:], in1=t2[:],
                                    op=ALU.subtract)            # m2
            nc.vector.tensor_tensor(out=t2[:], in0=usq[:, 2 * C:3 * C],
                                    in1=usq[:, 0:C], op=ALU.subtract)
            nc.scalar.mul(gf[:, 17 * C:18 * C], t2[:], cv[:, 15:16])  # m4
            nc.gpsimd.memset(gf[:, 18 * C:19 * C], 1.0)               # ones
            gfv = gf[:].rearrange("p (f c) -> p f c", f=NF)

            NSL = [(0, 512), (512, ET)] if ET > 512 else [(0, ET)]
            # ---------------- edge tiles (software-pipelined) ----------------
            # front(t): gathers + geometry transpose + radial MLP (PE+Act)
            # back(t-1): Sel/W4p tensor product + scatter (PE+DVE)
            state = {"win_ps": None}

            def stageB(t):
                wcols = slice(t * (NIW // NT), (t + 1) * (NIW // NT))
                aiS = sp.tile([P, ET], BF16, tag="aiS", bufs=3)
                nc.gpsimd.dma_gather(
                    aiS[:].unsqueeze(1), Tsb[:, :], isrc[:, wcols], ET, ET, P,
                    transpose=True, sbuf_tokens_per_rank=128,
                    sbuf_free_dim_per_rank=256)
                aiD = sp.tile([P, ET], BF16, tag="aiD")
                nc.gpsimd.dma_gather(
                    aiD[:].unsqueeze(1), Tsb[:, :], idst[:, wcols], ET, ET, P,
                    transpose=True, sbuf_tokens_per_rank=128,
                    sbuf_free_dim_per_rank=256)
                oht = sp.tile([P, ET], BF16, tag="oht", bufs=4)
                nc.sync.dma_start(oht[:], ohm_d[:, t * ET:(t + 1) * ET])
                # batched transpose of geometry features -> [19, ET]
                btp = pf.tile([NF, ET], BF16, tag="f", name="btp")
                for cc in range(TILE_CH):
                    cg = t * TILE_CH + cc
                    nc.tensor.transpose(btp[:, cc * P:(cc + 1) * P],
                                        gfv[:, 0:NF, cg], identb[:])
                bt19 = sp.tile([NF, ET], BF16, tag="bt19")
                nc.vector.tensor_copy(bt19[:], btp[:])
                # radial MLP layer 1 merged with sh replication (one lhsT)
                HP = ps.tile([124, ET], F32, tag="s", name="HP")
                for a, b in NSL:
                    nc.tensor.matmul(HP[:, a:b], HSt[:], bt19[:, a:b],
                                     start=True, stop=True)
                h1 = sp.tile([64, ET], BF16, tag="eh1")
                nc.scalar.activation(h1[:], HP[0:64, :], AF.Silu)
                shs = sp.tile([60, ET], BF16, tag="shs", bufs=3)
                nc.scalar.copy(shs[:], HP[64:124, :])
                h2p = ps.tile([64, ET], F32, tag="s")
                for a, b in NSL:
                    nc.tensor.matmul(h2p[:, a:b], fW2[:], h1[:, a:b],
                                     start=True, stop=True)
                h2 = sp.tile([64, ET], BF16, tag="eh2")
                nc.scalar.activation(h2[:], h2p[:], AF.Silu)
                return dict(t=t, aiS=aiS, aiD=aiD, oht=oht, shs=shs, h2=h2)

            def cps_partial(cps, m, q, rq):
                for a, b in NSL:
                    nc.tensor.matmul(cps[:, a:b],
                                     W4pt[:, q * 224 + m * 112:
                                          q * 224 + (m + 1) * 112],
                                     rq[:, a:b], start=(q == 0), stop=(q == 3))

            def stageC(cur):
                aiS, aiD, h2 = cur["aiS"], cur["aiD"], cur["h2"]
                # layer-3 matmul pre-expanded by Sel (silu commutes with the
                # 0/1 row selection); rq TT runs all-bf16 at 2x DVE rate;
                # W4p m=0 contraction interleaved per-quadrant
                rqs = []
                cps0 = None
                for q in range(4):
                    wrp = ps.tile([P, ET], F32, tag="s")
                    for a, b in NSL:
                        nc.tensor.matmul(wrp[:, a:b],
                                         fW3[:, 128 * q:128 * (q + 1)],
                                         h2[:, a:b], start=True, stop=True)
                    wrpS = sp.tile([P, ET], BF16, tag=f"wrpS{q}")
                    nc.scalar.activation(wrpS[:], wrp[:], AF.Silu)
                    rq = sp.tile([P, ET], BF16, tag=f"rq{q}")
                    nc.vector.tensor_tensor(out=rq[:], in0=wrpS[:], in1=aiD[:],
                                            op=ALU.mult)
                    rqs.append(rq)
                    if q == 1:
                        cps0 = pc.tile([112, ET], F32, tag="c", name="cps0")
                        cps_partial(cps0, 0, 0, rqs[0])
                        cps_partial(cps0, 0, 1, rqs[1])
                    elif q > 1:
                        cps_partial(cps0, 0, q, rq)
                tm0 = sp.tile([112, ET], BF16, tag="tm0")
                nc.vector.tensor_tensor(out=tm0[:], in0=cps0[:],
                                        in1=aiS[0:112, :], op=ALU.mult)
                cur["rqs"], cur["tm0"] = rqs, tm0

            def stageD(cur):
                aiS, shs = cur["aiS"], cur["shs"]
                cps1 = pc.tile([112, ET], F32, tag="c", name="cps1")
                for q in range(4):
                    cps_partial(cps1, 1, q, cur["rqs"][q])
                tm1 = sp.tile([112, ET], BF16, tag="tm1")
                nc.vector.tensor_tensor(out=tm1[:], in0=cps1[:],
                                        in1=aiS[0:112, :], op=ALU.mult)
                # contract u, map to 60-row output layout, x sh
                fps = ps.tile([60, ET], F32, tag="s")
                for a, b in NSL:
                    nc.tensor.matmul(fps[:, a:b], L2At[:], cur["tm0"][:, a:b],
                                     start=True, stop=False)
                    nc.tensor.matmul(fps[:, a:b], L2Bt[:], tm1[:, a:b],
                                     start=False, stop=True)
                F = sp.tile([60, ET], BF16, tag="F")
                nc.vector.tensor_tensor(out=F[:], in0=fps[:], in1=shs[:],
                                        op=ALU.mult)
                cur["F"] = F

            def stageE(cur):
                t, oht, F = cur["t"], cur["oht"], cur["F"]
                # scatter: transpose each chunk, one copy, PSUM-accum matmuls
                ftp = pf.tile([P, TILE_CH * 60], BF16, tag="f",
                              name="ftp")
                for cc in range(TILE_CH):
                    nc.tensor.transpose(ftp[:, cc * 60:(cc + 1) * 60],
                                        F[:, cc * P:(cc + 1) * P],
                                        identb[0:60, 0:60])
                fc = sp.tile([P, TILE_CH * 60], BF16, tag="fc")
                nc.vector.tensor_copy(fc[:], ftp[:])
                for cc in range(TILE_CH):
                    cg = t * TILE_CH + cc
                    win = cg // W_CH
                    if cg % W_CH == 0:
                        state["win_ps"] = pw.tile([P, 60], F32, tag="w",
                                                  name="win_ps")
                    nc.tensor.matmul(state["win_ps"][:],
                                     oht[:, cc * P:(cc + 1) * P],
                                     fc[:, cc * 60:(cc + 1) * 60],
                                     start=(cg % W_CH == 0),
                                     stop=(cg % W_CH == W_CH - 1))
                    if cg % W_CH == W_CH - 1:
                        wsb = sp.tile([P, 60], F32, tag="wsb")
                        nc.scalar.copy(wsb[:], state["win_ps"][:])
                        nc.sync.dma_start(out_d[win * P:(win + 1) * P, :],
                                          wsb[:])

            tiles = {}
            for i in range(NT + 3):
                if i < NT:
                    tiles[i] = stageB(i)
                if i - 1 >= 0 and i - 1 < NT:
                    stageC(tiles[i - 1])
                if i - 3 >= 0:
                    stageE(tiles.pop(i - 3))
                if i - 2 >= 0 and i - 2 < NT:
                    stageD(tiles[i - 2])
    nc.compile()
    return nc


_CACHE = {}


def kernel(**inputs):
    per_core, onehotA, W_CH, C_TOT, E = _host_prep(inputs)
    et = np.asarray(inputs["embed_table"], np.float32)
    fW4 = np.asarray(inputs["fc_W4"], np.float32)
    consts = _build_consts(fW4)
    W4p, Sel, L2A, L2B, HSb = consts
    HS = _merge_hs(HSb, np.asarray(inputs["fc_W1"], np.float32) / 1.12)
    key = (W_CH, C_TOT)
    if key not in _CACHE:
        _CACHE[key] = _build_bass(W_CH, C_TOT, E, consts)
    nc = _CACHE[key]
    shared = dict(
        onehotA=onehotA,
        TA=np.ascontiguousarray(
            (et @ np.asarray(inputs["fit_W1"], np.float32)).T),
        fit_W2=np.asarray(inputs["fit_W2"], np.float32),
        fit_W3=np.asarray(inputs["fit_W3"], np.float32),
        HS=HS,
        fc_W2p=(np.asarray(inputs["fc_W2"], np.float32) / 8.0),
        fc_W3p=np.ascontiguousarray(np.concatenate(
            [(np.asarray(inputs["fc_W3"], np.float32) / 8.0) @ Sel[q]
             for q in range(4)], axis=1)),
        W4p=np.ascontiguousarray(np.transpose(W4p.reshape(4, 128, 224), (1, 0, 2)).reshape(128, 896)),
        cvec=np.tile(np.array([*(-VCENTERS / STEP), 1.0 / STEP, -1.0,
                               15.0 ** 0.5, 0.5 * 5.0 ** 0.5, 5.0 ** 0.5,
                               0.5 * 15.0 ** 0.5], np.float32), (P, 1)),
        L2A=L2A, L2B=L2B,
    )
    import ml_dtypes
    for k in ("W4p", "L2A", "L2B", "HS", "TA", "fit_W2", "fit_W3",
              "fc_W2p", "fc_W3p", "onehotA"):
        shared[k] = shared[k].astype(ml_dtypes.bfloat16)
    in_maps = []
    for ci in range(N_CORES):
        m = dict(shared)
        m.update(geom_pl=per_core[ci]["geom_pl"],
                 oh_pl=per_core[ci]["oh_pl"].astype(ml_dtypes.bfloat16),
                 idx_src=per_core[ci]["idx_src"], idx_dst=per_core[ci]["idx_dst"])
        in_maps.append(m)
    res = bass_utils.run_bass_kernel_spmd(nc, in_maps, core_ids=list(range(N_CORES)))
    out = np.concatenate([res.results[ci]["out"][:NPC] for ci in range(N_CORES)], 0)
    return out.astype(np.float32)
